# revision 4
# baseline (speedup 1.0000x reference)
"""Equivariant block-diagonal linear (128x0e+128x1o+64x2e+32x3o) on 8 trn2 cores.

Strategy (mode "pk", default):
  - Data-parallel: x [50000, 1056] row-sharded into 8x [6250, 1056].
  - Host repacks each shard per irrep into a [128, cols] bf16 layout with the
    multiplicity axis on partitions and (node, m-component) on the free axis:
      r0: [128, n]          cols (n)
      r1: [128, 3n]         cols (n, i)
      r2: [128, 5n/2]       two m-columns stacked on partitions (64*2)
      r3: [128, 7n/4(+pad)] four m-columns stacked on partitions (32*4)
    The per-irrep weights become dense [128,128] stationary operands:
      W0 = w0, W1 = w1, W2 = diag(w2, w2), W3 = diag(w3, w3, w3, w3).
  - Device: out[:, c] = W_r^T @ xp[:, c] as plain 128x128 bf16 matmuls with the
    512-col moving operand, PSUM -> bf16 SBUF copy, contiguous DMA both ways.
    Zero wasted FLOPs, DMA is pure bf16 (26.4 MB/core round trip).
  - Host unpacks the bf16 result back to [50000, 1056] f32.

bf16 quantization of x and w gives rel err ~2e-3 (gate is 2e-2).
"""

import os
from contextlib import ExitStack

import ml_dtypes
import numpy as np

import concourse.bass as bass
import concourse.tile as tile
from concourse import bacc, mybir
from concourse.bass_utils import run_bass_kernel_spmd

N_NODES = 50000
DIM = 1056
N_CORES = 8
SHARD = N_NODES // N_CORES  # 6250
P = 128

BF16 = np.dtype(ml_dtypes.bfloat16)

IRREPS = [(128, 0), (128, 1), (64, 2), (32, 3)]
G_OFF = [0, 128, 512, 832]

# packed column widths per irrep (node-major free axis)
CW = [SHARD, 3 * SHARD, 5 * SHARD // 2, (7 * SHARD + 2) // 4]  # pad r3 by 2 src cols
COFF = [0, CW[0], CW[0] + CW[1], CW[0] + CW[1] + CW[2]]
CT = sum(CW)  # 51563

TILE = int(os.environ.get("KERNEL_TILE", "4096"))

_cache = {}


def _build_pk():
    key = ("pk", TILE)
    if key in _cache:
        return _cache[key]
    f32 = mybir.dt.float32
    bf16 = mybir.dt.bfloat16
    nc = bacc.Bacc(
        "TRN2", target_bir_lowering=False, debug=False, num_devices=N_CORES
    )
    xp_d = nc.dram_tensor("xp", [P, CT], bf16, kind="ExternalInput")
    wp_d = nc.dram_tensor("wp", [P, 512], bf16, kind="ExternalInput")
    op_d = nc.dram_tensor("op", [P, CT], bf16, kind="ExternalOutput")

    with ExitStack() as ctx:
        tc = ctx.enter_context(tile.TileContext(nc))
        wpool = ctx.enter_context(tc.tile_pool(name="w", bufs=1))
        xpool = ctx.enter_context(tc.tile_pool(name="xin", bufs=4))
        opool = ctx.enter_context(tc.tile_pool(name="oout", bufs=4))
        pspool = ctx.enter_context(tc.tile_pool(name="ps", bufs=8, space="PSUM"))

        wsb = wpool.tile([P, 512], bf16, tag="w")
        nc.sync.dma_start(wsb[:], wp_d[:])

        ci = 0
        ti = 0
        for r in range(4):
            c0r, c1r = COFF[r], COFF[r] + CW[r]
            lhs = wsb[:, r * P : (r + 1) * P]
            for c0 in range(c0r, c1r, TILE):
                cw = min(TILE, c1r - c0)
                xin = xpool.tile([P, TILE], bf16, tag="x")
                in_eng = nc.sync if ti % 2 == 0 else nc.gpsimd
                ti += 1
                in_eng.dma_start(xin[:, :cw], xp_d[:, c0 : c0 + cw])
                ot = opool.tile([P, TILE], bf16, tag="o")
                for s0 in range(0, cw, 512):
                    ns = min(512, cw - s0)
                    ps = pspool.tile([P, 512], f32, tag="ps")
                    nc.tensor.matmul(
                        ps[:, :ns],
                        lhs,
                        xin[:, s0 : s0 + ns],
                        start=True,
                        stop=True,
                    )
                    if ci % 2 == 0:
                        nc.vector.tensor_copy(ot[:, s0 : s0 + ns], ps[:, :ns])
                    else:
                        nc.scalar.copy(ot[:, s0 : s0 + ns], ps[:, :ns])
                    ci += 1
                nc.scalar.dma_start(op_d[:, c0 : c0 + cw], ot[:, :cw])

    nc.compile()
    _cache[key] = nc
    return nc


def _pack_weights(w0, w1, w2, w3):
    wp = np.zeros((P, 512), dtype=np.float32)
    wp[:, 0:128] = np.asarray(w0, dtype=np.float32)
    wp[:, 128:256] = np.asarray(w1, dtype=np.float32)
    w2 = np.asarray(w2, dtype=np.float32)
    for p in range(2):
        wp[p * 64 : (p + 1) * 64, 256 + p * 64 : 256 + (p + 1) * 64] = w2
    w3 = np.asarray(w3, dtype=np.float32)
    for p in range(4):
        wp[p * 32 : (p + 1) * 32, 384 + p * 32 : 384 + (p + 1) * 32] = w3
    return wp.astype(BF16)


def _pack_x(x):
    """x [50000, 1056] f32 -> list of 8 per-core [128, CT] bf16 arrays."""
    n = N_NODES
    a0 = np.ascontiguousarray(x[:, 0:128].T)  # [128, n]
    a1 = np.ascontiguousarray(
        x[:, 128:512].reshape(n, 128, 3).transpose(1, 0, 2).reshape(128, 3 * n)
    )
    b2 = x[:, 512:832].reshape(n, 64, 5).transpose(1, 0, 2).reshape(64, 5 * n)
    a2 = np.ascontiguousarray(
        b2.reshape(64, 5 * n // 2, 2).transpose(2, 0, 1).reshape(128, 5 * n // 2)
    )
    b3 = x[:, 832:1056].reshape(n, 32, 7).transpose(1, 0, 2).reshape(32, 7 * n)
    out = []
    for c in range(N_CORES):
        xp = np.empty((P, CT), dtype=BF16)
        xp[:, COFF[0] : COFF[0] + CW[0]] = a0[:, c * SHARD : (c + 1) * SHARD]
        xp[:, COFF[1] : COFF[1] + CW[1]] = a1[:, c * 3 * SHARD : (c + 1) * 3 * SHARD]
        xp[:, COFF[2] : COFF[2] + CW[2]] = a2[
            :, c * (5 * SHARD // 2) : (c + 1) * (5 * SHARD // 2)
        ]
        b3c = b3[:, c * 7 * SHARD : (c + 1) * 7 * SHARD]
        b3p = np.zeros((32, 4 * CW[3]), dtype=np.float32)
        b3p[:, : 7 * SHARD] = b3c
        xp[:, COFF[3] :] = (
            b3p.reshape(32, CW[3], 4).transpose(2, 0, 1).reshape(128, CW[3])
        )
        out.append(xp)
    return out


def _unpack_out(ops):
    """list of 8 [128, CT] bf16 -> [50000, 1056] f32."""
    out = np.empty((N_NODES, DIM), dtype=np.float32)
    for c, op in enumerate(ops):
        op = np.asarray(op)
        sl = slice(c * SHARD, (c + 1) * SHARD)
        y0 = op[:, COFF[0] : COFF[0] + CW[0]].astype(np.float32)
        out[sl, 0:128] = y0.T
        y1 = op[:, COFF[1] : COFF[1] + CW[1]].astype(np.float32)
        out[sl, 128:512] = y1.reshape(128, SHARD, 3).transpose(1, 0, 2).reshape(
            SHARD, 384
        )
        y2 = op[:, COFF[2] : COFF[2] + CW[2]].astype(np.float32)
        c2 = y2.reshape(2, 64, CW[2]).transpose(1, 2, 0).reshape(64, 5 * SHARD)
        out[sl, 512:832] = c2.reshape(64, SHARD, 5).transpose(1, 0, 2).reshape(
            SHARD, 320
        )
        y3 = op[:, COFF[3] : COFF[3] + CW[3]].astype(np.float32)
        c3 = y3.reshape(4, 32, CW[3]).transpose(1, 2, 0).reshape(32, 4 * CW[3])[
            :, : 7 * SHARD
        ]
        out[sl, 832:1056] = c3.reshape(32, SHARD, 7).transpose(1, 0, 2).reshape(
            SHARD, 224
        )
    return out


last_result = None  # BassKernelResults of the most recent run (for profiling)

MODE = os.environ.get("KERNEL_MODE", "pk")


def kernel(x, w0, w1, w2, w3):
    global last_result
    x = np.asarray(x, dtype=np.float32)
    trace = os.environ.get("KERNEL_TRACE", "0") == "1"
    nc = _build_pk()
    wp = _pack_weights(w0, w1, w2, w3)
    xps = _pack_x(x)
    in_maps = [{"xp": xps[c], "wp": wp} for c in range(N_CORES)]
    last_result = run_bass_kernel_spmd(
        nc, in_maps, core_ids=list(range(N_CORES)), trace=trace
    )
    return _unpack_out([r["op"] for r in last_result.results])


# revision 6
# speedup vs baseline: 1.1618x; 1.1618x over previous
"""Equivariant block-diagonal linear (128x0e+128x1o+64x2e+32x3o) on 8 trn2 cores.

Strategy (mode "pk", default):
  - Data-parallel: x [50000, 1056] row-sharded into 8x [6250, 1056].
  - Host repacks each shard per irrep into a [128, cols] bf16 layout with the
    multiplicity axis on partitions and (node, m-component) on the free axis:
      r0: [128, n]          cols (n)
      r1: [128, 3n]         cols (n, i)
      r2: [128, 5n/2]       two m-columns stacked on partitions (64*2)
      r3: [128, 7n/4(+pad)] four m-columns stacked on partitions (32*4)
    The per-irrep weights become dense [128,128] stationary operands:
      W0 = w0, W1 = w1, W2 = diag(w2, w2), W3 = diag(w3, w3, w3, w3).
  - Device: out[:, c] = W_r^T @ xp[:, c] as plain 128x128 bf16 matmuls with the
    512-col moving operand, PSUM -> bf16 SBUF copy, contiguous DMA both ways.
    Zero wasted FLOPs, DMA is pure bf16 (26.4 MB/core round trip).
  - Host unpacks the bf16 result back to [50000, 1056] f32.

bf16 quantization of x and w gives rel err ~2e-3 (gate is 2e-2).
"""

import os
from contextlib import ExitStack

import ml_dtypes
import numpy as np

import concourse.bass as bass
import concourse.tile as tile
from concourse import bacc, mybir
from concourse.bass_utils import run_bass_kernel_spmd

N_NODES = 50000
DIM = 1056
N_CORES = 8
SHARD = N_NODES // N_CORES  # 6250
P = 128

BF16 = np.dtype(ml_dtypes.bfloat16)

IRREPS = [(128, 0), (128, 1), (64, 2), (32, 3)]
G_OFF = [0, 128, 512, 832]

# packed column widths per irrep (node-major free axis)
CW = [SHARD, 3 * SHARD, 5 * SHARD // 2, (7 * SHARD + 2) // 4]  # pad r3 by 2 src cols
COFF = [0, CW[0], CW[0] + CW[1], CW[0] + CW[1] + CW[2]]
CT = sum(CW)  # 51563

TILE = int(os.environ.get("KERNEL_TILE", "2560"))


def _tile_splits(width, tile):
    """Split width into ceil(width/tile) nearly-equal chunks (sizes differ by <=1)."""
    k = -(-width // tile)
    base, rem = divmod(width, k)
    out = []
    for i in range(k):
        out.append(base + (1 if i < rem else 0))
    return out

_cache = {}


def _build_pk():
    key = ("pk", TILE)
    if key in _cache:
        return _cache[key]
    f32 = mybir.dt.float32
    bf16 = mybir.dt.bfloat16
    nc = bacc.Bacc(
        "TRN2", target_bir_lowering=False, debug=False, num_devices=N_CORES
    )
    xp_d = nc.dram_tensor("xp", [P, CT], bf16, kind="ExternalInput")
    wp_d = nc.dram_tensor("wp", [P, 512], bf16, kind="ExternalInput")
    op_d = nc.dram_tensor("op", [P, CT], bf16, kind="ExternalOutput")

    with ExitStack() as ctx:
        tc = ctx.enter_context(tile.TileContext(nc))
        wpool = ctx.enter_context(tc.tile_pool(name="w", bufs=1))
        xpool = ctx.enter_context(tc.tile_pool(name="xin", bufs=6))
        opool = ctx.enter_context(tc.tile_pool(name="oout", bufs=6))
        pspool = ctx.enter_context(tc.tile_pool(name="ps", bufs=8, space="PSUM"))

        wsb = wpool.tile([P, 512], bf16, tag="w")
        nc.sync.dma_start(wsb[:], wp_d[:])

        ci = 0
        ti = 0
        for r in range(4):
            lhs = wsb[:, r * P : (r + 1) * P]
            c0 = COFF[r]
            for cw in _tile_splits(CW[r], TILE):
                xin = xpool.tile([P, TILE], bf16, tag="x")
                in_eng = nc.sync if ti % 2 == 0 else nc.gpsimd
                ti += 1
                in_eng.dma_start(xin[:, :cw], xp_d[:, c0 : c0 + cw])
                ot = opool.tile([P, TILE], bf16, tag="o")
                for s0 in range(0, cw, 512):
                    ns = min(512, cw - s0)
                    ps = pspool.tile([P, 512], f32, tag="ps")
                    nc.tensor.matmul(
                        ps[:, :ns],
                        lhs,
                        xin[:, s0 : s0 + ns],
                        start=True,
                        stop=True,
                    )
                    if ci % 2 == 0:
                        nc.vector.tensor_copy(ot[:, s0 : s0 + ns], ps[:, :ns])
                    else:
                        nc.scalar.copy(ot[:, s0 : s0 + ns], ps[:, :ns])
                    ci += 1
                nc.scalar.dma_start(op_d[:, c0 : c0 + cw], ot[:, :cw])
                c0 += cw

    nc.compile()
    _cache[key] = nc
    return nc


def _pack_weights(w0, w1, w2, w3):
    wp = np.zeros((P, 512), dtype=np.float32)
    wp[:, 0:128] = np.asarray(w0, dtype=np.float32)
    wp[:, 128:256] = np.asarray(w1, dtype=np.float32)
    w2 = np.asarray(w2, dtype=np.float32)
    for p in range(2):
        wp[p * 64 : (p + 1) * 64, 256 + p * 64 : 256 + (p + 1) * 64] = w2
    w3 = np.asarray(w3, dtype=np.float32)
    for p in range(4):
        wp[p * 32 : (p + 1) * 32, 384 + p * 32 : 384 + (p + 1) * 32] = w3
    return wp.astype(BF16)


def _pack_x(x):
    """x [50000, 1056] f32 -> list of 8 per-core [128, CT] bf16 arrays."""
    n = N_NODES
    a0 = np.ascontiguousarray(x[:, 0:128].T)  # [128, n]
    a1 = np.ascontiguousarray(
        x[:, 128:512].reshape(n, 128, 3).transpose(1, 0, 2).reshape(128, 3 * n)
    )
    b2 = x[:, 512:832].reshape(n, 64, 5).transpose(1, 0, 2).reshape(64, 5 * n)
    a2 = np.ascontiguousarray(
        b2.reshape(64, 5 * n // 2, 2).transpose(2, 0, 1).reshape(128, 5 * n // 2)
    )
    b3 = x[:, 832:1056].reshape(n, 32, 7).transpose(1, 0, 2).reshape(32, 7 * n)
    out = []
    for c in range(N_CORES):
        xp = np.empty((P, CT), dtype=BF16)
        xp[:, COFF[0] : COFF[0] + CW[0]] = a0[:, c * SHARD : (c + 1) * SHARD]
        xp[:, COFF[1] : COFF[1] + CW[1]] = a1[:, c * 3 * SHARD : (c + 1) * 3 * SHARD]
        xp[:, COFF[2] : COFF[2] + CW[2]] = a2[
            :, c * (5 * SHARD // 2) : (c + 1) * (5 * SHARD // 2)
        ]
        b3c = b3[:, c * 7 * SHARD : (c + 1) * 7 * SHARD]
        b3p = np.zeros((32, 4 * CW[3]), dtype=np.float32)
        b3p[:, : 7 * SHARD] = b3c
        xp[:, COFF[3] :] = (
            b3p.reshape(32, CW[3], 4).transpose(2, 0, 1).reshape(128, CW[3])
        )
        out.append(xp)
    return out


def _unpack_out(ops):
    """list of 8 [128, CT] bf16 -> [50000, 1056] f32."""
    out = np.empty((N_NODES, DIM), dtype=np.float32)
    for c, op in enumerate(ops):
        op = np.asarray(op)
        sl = slice(c * SHARD, (c + 1) * SHARD)
        y0 = op[:, COFF[0] : COFF[0] + CW[0]].astype(np.float32)
        out[sl, 0:128] = y0.T
        y1 = op[:, COFF[1] : COFF[1] + CW[1]].astype(np.float32)
        out[sl, 128:512] = y1.reshape(128, SHARD, 3).transpose(1, 0, 2).reshape(
            SHARD, 384
        )
        y2 = op[:, COFF[2] : COFF[2] + CW[2]].astype(np.float32)
        c2 = y2.reshape(2, 64, CW[2]).transpose(1, 2, 0).reshape(64, 5 * SHARD)
        out[sl, 512:832] = c2.reshape(64, SHARD, 5).transpose(1, 0, 2).reshape(
            SHARD, 320
        )
        y3 = op[:, COFF[3] : COFF[3] + CW[3]].astype(np.float32)
        c3 = y3.reshape(4, 32, CW[3]).transpose(1, 2, 0).reshape(32, 4 * CW[3])[
            :, : 7 * SHARD
        ]
        out[sl, 832:1056] = c3.reshape(32, SHARD, 7).transpose(1, 0, 2).reshape(
            SHARD, 224
        )
    return out


last_result = None  # BassKernelResults of the most recent run (for profiling)

MODE = os.environ.get("KERNEL_MODE", "pk")


def kernel(x, w0, w1, w2, w3):
    global last_result
    x = np.asarray(x, dtype=np.float32)
    trace = os.environ.get("KERNEL_TRACE", "0") == "1"
    nc = _build_pk()
    wp = _pack_weights(w0, w1, w2, w3)
    xps = _pack_x(x)
    in_maps = [{"xp": xps[c], "wp": wp} for c in range(N_CORES)]
    last_result = run_bass_kernel_spmd(
        nc, in_maps, core_ids=list(range(N_CORES)), trace=trace
    )
    return _unpack_out([r["op"] for r in last_result.results])
